# revision 1
# baseline (speedup 1.0000x reference)
"""Trainium2 Bass kernel for nn_DiagonalMatrixModel.

Reference computes out[i, j] = logsumexp_k(A[i, k] + x[k, j]) with
A = diag(d): a dense log-domain matmul with a diagonal left operand.
Because A[i, k] = d[i] if k == i else 0, the logsumexp collapses exactly:

    out[i, j] = log( sum_{k != i} exp(x[k, j]) + exp(d[i] + x[i, j]) )
              = log( S[j] + exp(x[i, j]) * w[i] ),   w = exp(d) - 1,
    S[j] = sum_k exp(x[k, j])

i.e. O(N^2) work instead of the reference's O(N^3). w is a pure
transform of the learned parameter d, so it is folded on the host
(standard weight preprocessing), keeping the device path x -> out.

Sharding: x and out are split along the column axis j across 8 cores
(64 columns each); w is replicated. Each core computes its S[j]
locally -- no cross-device communication.

Per-core layout: the [512, 64] column shard is viewed as [128, 256]
(partition p holds rows 4p..4p+3); w[4p:4p+4] plus 1.0/0.0 constants are
packed into the same host-side buffer, so each partition's input bytes
are contiguous and ONE DMA fetches everything (and every on-chip
dependency hangs off that single DMA semaphore). The cross-partition
sum S is computed on the tensor engine with an all-ones stationary
matrix (f32r rate), which also broadcasts S across all 128 partitions
of the PSUM accumulator for free.
"""

import types

import numpy as np

import bass_rust
import concourse.bacc as bacc
import concourse.bass as bass
import concourse.mybir as mybir
from concourse import tile
from concourse.bass import ts
from concourse.bass_utils import run_bass_kernel_spmd
from concourse.hw_specs import get_activation_tables

N_CORES = 8
SIZE = 512          # rows (k / i axis)
N_COLS = 512        # full column count
J = N_COLS // N_CORES  # columns per core
P = 128             # SBUF partitions
R = SIZE // P       # row blocks per partition (4)
F = R * J           # x free-dim elements per partition (256)
FW = F + R + 2      # packed free dim: x (256) + w (4) + consts 1.0, 0.0
HF = F // 2         # half of the x free dim (128)

FP32 = mybir.dt.float32
F32R = mybir.dt.float32r
Exp = mybir.ActivationFunctionType.Exp
Ln = mybir.ActivationFunctionType.Ln

# The default act-table chooser greedily picks the first set containing
# each needed function (exp_and_others for Exp, then natural_log for Ln)
# => two ~1.3us LoadActFuncSet ops. natural_log_exp_and_others contains
# every function this kernel uses, so blank out all other sets (keeping
# list positions, which define act_func_set_id) to force ONE table load.
_COMBINED_SET = "natural_log_exp_and_others"


def _patched_insert_act_table_loads(self):
    has_activation = any(
        isinstance(i, mybir.InstActivation)
        for b in self.main_func.blocks
        for i in b.instructions
    )
    if not has_activation:
        return
    all_tables = get_activation_tables(self.m.arch)
    if _COMBINED_SET in all_tables:
        tables = [
            (name, funcs if name == _COMBINED_SET else set())
            for name, funcs in all_tables.items()
        ]
    else:  # safety: unknown act_info layout -> default behavior
        tables = list(all_tables.items())
    bass_rust.insert_act_table_loads(self, tables)


def _strip_const_preamble(nc) -> None:
    """Drop the const-AP preamble: the 4 memsets and the all-engine
    barrier that publishes them. This kernel passes its own zeros tile as
    the activation bias, so no const AP is ever read. Saves ~600ns before
    the input DMA can issue."""
    bb = nc.main_func.blocks[0]
    dead = [
        ins
        for ins in bb.instructions
        if type(ins).__name__ in ("InstMemset", "InstDrain", "InstEventSemaphore")
    ]
    for ins in dead:
        bb.instructions.remove(ins)


def _strip_post_clear_barrier(nc) -> None:
    """Drop the all-engine barrier emitted AFTER the kernel-tail semaphore
    clear. NEFF completion requires every engine stream to end, and the
    Pool sem-clear is Pool's last instruction either way, so the barrier
    only delays stream-end by ~300ns. Sem state for re-execution is
    unchanged (the clear itself is kept, ordered after the pre-clear
    barrier)."""
    bb = nc.main_func.blocks[-1]
    isa_idx = max(
        (i for i, ins in enumerate(bb.instructions)
         if type(ins).__name__ == "InstISA"),
        default=None,
    )
    if isa_idx is None:
        return
    tail = bb.instructions[isa_idx + 1 :]
    if not all(
        type(ins).__name__ in ("InstDrain", "InstEventSemaphore") for ins in tail
    ):
        return  # unexpected tail layout -> leave it intact
    for ins in tail:
        bb.instructions.remove(ins)


def build_kernel() -> bass.Bass:
    nc = bacc.Bacc("TRN2")
    nc.insert_act_table_loads = types.MethodType(_patched_insert_act_table_loads, nc)
    _strip_const_preamble(nc)

    xd = nc.dram_tensor("xd", [P, FW], FP32, kind="ExternalInput")
    out = nc.dram_tensor("out", [SIZE, J], FP32, kind="ExternalOutput")
    out_v = out[:].rearrange("(p r) j -> p (r j)", p=P)  # [128, 256]

    with tile.TileContext(nc) as tc:
        with (
            tc.tile_pool(name="sbuf", bufs=1) as sbuf,
            tc.tile_pool(name="psum", bufs=1, space="PSUM") as psum,
        ):
            xt = sbuf.tile([P, FW], FP32)
            ones = sbuf.tile([P, P], F32R)

            # Single input DMA: consecutive transfers complete ~380ns
            # apart (HWDGE FIFO + DGE delay) which exceeds what a split
            # could hide, so one contiguous transfer wins.
            nc.sync.dma_start(xt[:], xd[:])
            w = xt[:, F : F + R]               # packed exp(diag)-1, [128, 4]
            one_col = xt[:, F + R : F + R + 1]   # packed 1.0 column
            zeros = xt[:, F + R + 1 : F + R + 2]  # packed 0.0 column
            # f32r ones for the PE: memset can't emit f32r, so broadcast-copy
            # the packed 1.0 column through the (otherwise idle) DVE. Using
            # packed constants keeps every ACT/DVE dependency on the one DMA
            # semaphore -- no cross-engine preamble, no event-split stalls.
            nc.vector.tensor_copy(ones[:], one_col.to_broadcast((P, P)))

            # E = exp(x). Produced as f32r (f32 bits with the PE's reduced
            # mantissa rounding) so the matmuls can run at the f32r rate;
            # worst case ~1e-4 relative rounding, far inside tolerance.
            E = sbuf.tile([P, F], F32R)
            nc.scalar.activation(E[:, 0:HF], xt[:, 0:HF], Exp, bias=zeros)
            nc.scalar.activation(E[:, HF:F], xt[:, HF:F], Exp, bias=zeros)

            # B[m, j] = S[j] for all m: ones.T @ E accumulated over row
            # blocks; f32r runs the PE at 2-4x the f32 rate.
            B = psum.tile([P, J], FP32)
            for t in range(R):
                nc.tensor.matmul(
                    B[:],
                    ones[:],
                    E[:, ts(t, J)],
                    start=(t == 0),
                    stop=(t == R - 1),
                )

            # tmp = E * w + S. The multiply is split in halves so each
            # half starts right after its exp half and DVE's per-op drain
            # finishes before B's semaphore arrives for the add.
            tmp = sbuf.tile([P, F], FP32)
            t3 = tmp[:].rearrange("p (r j) -> p r j", r=R)
            RH = R // 2
            for h in range(2):
                nc.vector.tensor_tensor(
                    tmp[:, h * HF : (h + 1) * HF].rearrange(
                        "p (r j) -> p r j", r=RH
                    ),
                    E[:, h * HF : (h + 1) * HF]
                    .bitcast(FP32)
                    .rearrange("p (r j) -> p r j", r=RH),
                    w[:, h * RH : (h + 1) * RH, None].to_broadcast((P, RH, J)),
                    op=mybir.AluOpType.mult,
                )
            nc.vector.tensor_tensor(
                t3,
                t3,
                B[:, None, :].to_broadcast((P, R, J)),
                op=mybir.AluOpType.add,
            )

            # out = log(tmp); single full-width Ln + one output DMA on SP
            res = sbuf.tile([P, F], FP32)
            nc.scalar.activation(res[:], tmp[:], Ln, bias=zeros)
            nc.sync.dma_start(out_v, res[:])

    _strip_post_clear_barrier(nc)
    nc.compile()
    return nc


_NC_CACHE = None


def _pack_inputs(x: np.ndarray, diag: np.ndarray) -> list[dict[str, np.ndarray]]:
    w = np.exp(diag.astype(np.float64)).astype(np.float32) - 1.0
    w_blocks = w.reshape(P, R)  # w[4p + r]
    in_maps = []
    for c in range(N_CORES):
        shard = x[:, c * J : (c + 1) * J]           # [512, 64]
        xd = np.empty((P, FW), dtype=np.float32)
        xd[:, 0:F] = shard.reshape(P, F)            # rows 4p..4p+3 -> partition p
        xd[:, F : F + R] = w_blocks
        xd[:, F + R] = 1.0
        xd[:, F + R + 1] = 0.0
        in_maps.append({"xd": xd})
    return in_maps


def kernel(x: np.ndarray, diag: np.ndarray, trace: bool = False):
    global _NC_CACHE
    if _NC_CACHE is None:
        _NC_CACHE = build_kernel()
    nc = _NC_CACHE

    x = np.ascontiguousarray(np.asarray(x, dtype=np.float32))
    diag = np.asarray(diag, dtype=np.float32)

    in_maps = _pack_inputs(x, diag)
    res = run_bass_kernel_spmd(nc, in_maps, core_ids=list(range(N_CORES)), trace=trace)
    full = np.concatenate([r["out"] for r in res.results], axis=1)
    if trace:
        return full, res
    return full



# revision 18
# speedup vs baseline: 1.2877x; 1.2877x over previous
"""Trainium2 Bass kernel for nn_DiagonalMatrixModel.

Reference computes out[i, j] = logsumexp_k(A[i, k] + x[k, j]) with
A = diag(d): a dense log-domain matmul with a diagonal left operand.
Because A[i, k] = d[i] if k == i else 0, the logsumexp collapses exactly:

    out[i, j] = log( sum_{k != i} exp(x[k, j]) + exp(d[i] + x[i, j]) )
              = log( S[j] + exp(x[i, j]) * w[i] ),   w = exp(d) - 1,
    S[j] = sum_k exp(x[k, j])

i.e. O(N^2) work instead of the reference's O(N^3). w is a pure
transform of the learned parameter d, so it is folded on the host
(standard weight preprocessing), keeping the device path x -> out.

Sharding: x and out are split along the column axis j across 8 cores
(64 columns each); w is replicated. Each core computes its S[j]
locally -- no cross-device communication.

Per-core layout: the [512, 64] column shard is viewed as [128, 256]
(partition p holds rows 4p..4p+3); w[4p:4p+4] plus a 0.0 constant are
packed into the same host-side buffer, so each partition's input bytes
are contiguous and ONE DMA fetches everything. The whole on-chip path
runs in fp16 (relative tolerance is 2e-2; fp16 keeps it ~1e-3): this
halves both DMA transfers and lets the DVE run in its packed-2-byte
fast mode and the PE at the 1-cycle/row fp16 rate. The cross-partition
sum S is computed on the tensor engine with an all-ones stationary
matrix, broadcasting S across all 128 partitions of PSUM for free; the
per-row weight multiply is four tensor_scalar ops (w[4p+r] is a
per-partition scalar within each row-block r), and Pool (otherwise
idle) rounds S to an SBUF fp16 copy so the final add runs in the DVE
fast mode too.
"""

import types

import numpy as np

import bass_rust
import concourse.bacc as bacc
import concourse.bass as bass
import concourse.mybir as mybir
from concourse import tile
from concourse.bass import ts
from concourse.bass_utils import run_bass_kernel_spmd
from concourse.hw_specs import get_activation_tables

N_CORES = 8
SIZE = 512          # rows (k / i axis)
N_COLS = 512        # full column count
J = N_COLS // N_CORES  # columns per core
P = 128             # SBUF partitions
R = SIZE // P       # row blocks per partition (4)
F = R * J           # x free-dim elements per partition (256)
FW = F + R + 2      # packed free dim: x (256) + w (4) + consts 1.0, 0.0
HF = F // 2         # half of the x free dim (128)

FP16 = mybir.dt.float16
FP32 = mybir.dt.float32
Exp = mybir.ActivationFunctionType.Exp
Ln = mybir.ActivationFunctionType.Ln
Copy = mybir.ActivationFunctionType.Copy

# The default act-table chooser greedily picks the first set containing
# each needed function (exp_and_others for Exp, then natural_log for Ln)
# => two ~1.3us LoadActFuncSet ops. natural_log_exp_and_others contains
# every function this kernel uses, so blank out all other sets (keeping
# list positions, which define act_func_set_id) to force ONE table load.
_COMBINED_SET = "natural_log_exp_and_others"


def _patched_insert_act_table_loads(self):
    has_activation = any(
        isinstance(i, mybir.InstActivation)
        for b in self.main_func.blocks
        for i in b.instructions
    )
    if not has_activation:
        return
    all_tables = get_activation_tables(self.m.arch)
    if _COMBINED_SET in all_tables:
        tables = [
            (name, funcs if name == _COMBINED_SET else set())
            for name, funcs in all_tables.items()
        ]
    else:  # safety: unknown act_info layout -> default behavior
        tables = list(all_tables.items())
    bass_rust.insert_act_table_loads(self, tables)


def _strip_const_preamble(nc) -> None:
    """Drop the const-AP preamble: the 4 memsets and the all-engine
    barrier that publishes them. This kernel passes its own zeros tile as
    the activation bias, so no const AP is ever read. Saves ~600ns before
    the input DMA can issue."""
    bb = nc.main_func.blocks[0]
    dead = [
        ins
        for ins in bb.instructions
        if type(ins).__name__ in ("InstMemset", "InstDrain", "InstEventSemaphore")
    ]
    for ins in dead:
        bb.instructions.remove(ins)


def _strip_post_clear_barrier(nc) -> None:
    """Drop the all-engine barrier emitted AFTER the kernel-tail semaphore
    clear. NEFF completion requires every engine stream to end, and the
    Pool sem-clear is Pool's last instruction either way, so the barrier
    only delays stream-end by ~300ns. Sem state for re-execution is
    unchanged (the clear itself is kept, ordered after the pre-clear
    barrier)."""
    bb = nc.main_func.blocks[-1]
    isa_idx = max(
        (i for i, ins in enumerate(bb.instructions)
         if type(ins).__name__ == "InstISA"),
        default=None,
    )
    if isa_idx is None:
        return
    tail = bb.instructions[isa_idx + 1 :]
    if not all(
        type(ins).__name__ in ("InstDrain", "InstEventSemaphore") for ins in tail
    ):
        return  # unexpected tail layout -> leave it intact
    for ins in tail:
        bb.instructions.remove(ins)


# Add variant: "dve_copy" = DVE copies S to SBUF fp16 then adds in fast
# mode (in-order, no extra sem hop); "psum" = DVE adds the PSUM f32
# accumulator directly in one slower op.
ADD_VIA = "psum"


def _retarget_writeback_sem(nc) -> None:
    """Point the kv_writeback prep's DMA-completion update at the builtin
    DMASW0 queue semaphore. Tile schedules the prep on the DMASW0 proc lane
    and makes downstream waiters (the kernel-tail barriers) wait
    DMASW0 >= 16, but the descriptor-baked sem comes from the user `sem=`
    kwarg -- without this rewrite the completion bumps the wrong sem and
    the tail deadlocks."""
    target = None
    for bb in nc.main_func.blocks:
        for ins in bb.instructions:
            si = getattr(ins, "sync_info", None)
            if not si:
                continue
            for w in si.on_wait:
                if w.ant_name and w.ant_name.startswith("DMASW0"):
                    target = (w.id, w.ant_name)
    assert target is not None, "no DMASW0 waiter found"
    for bb in nc.main_func.blocks:
        for ins in bb.instructions:
            if type(ins).__name__ == "InstKVWritebackAnt":
                upd = ins.sync_info.on_update[0]
                assert upd.ant_name == "out_wb_dma", upd.ant_name
                upd.id, upd.ant_name = target
                return
    raise AssertionError("kv_writeback prep not found")


def _strip_spurious_war_guards(nc) -> None:
    """Remove the write-after-read guards Tile places before the Ln and the
    trigger. The kv_writeback prep is emitted before res has a producer, so
    Tile models the prep's deferred res-read as completing at DMASW0>=16 and
    makes the later res writer (Ln) -- and even the trigger itself -- wait
    for it. The DMA only fires at the trigger, which already waits on the
    Ln via signals_writable, so these guards are a false cycle: the real
    ordering Ln -> trigger -> DMA is intact without them. The SP kernel-tail
    gate (which also waits DMASW0>=16, together with other sems) is kept --
    it is what holds the NEFF open until the output lands in DRAM."""
    for bb in nc.main_func.blocks:
        for ins in bb.instructions:
            if type(ins).__name__ not in ("InstActivation", "InstTriggerDma"):
                continue
            si = getattr(ins, "sync_info", None)
            if not si:
                continue
            kept = [
                w
                for w in si.on_wait
                if not (w.ant_name and w.ant_name.startswith("DMASW0"))
            ]
            if len(kept) != len(si.on_wait):
                si.on_wait = kept


def build_kernel() -> bass.Bass:
    nc = bacc.Bacc("TRN2")
    nc.insert_act_table_loads = types.MethodType(_patched_insert_act_table_loads, nc)
    _strip_const_preamble(nc)

    xd = nc.dram_tensor("xd", [P, FW], FP16, kind="ExternalInput")
    out = nc.dram_tensor("out", [SIZE, J], FP16, kind="ExternalOutput")
    # kv_writeback layout: dst[b, dhi, dho, ctx:ctx+ncn] = src[dhi, dho, b, :].
    # With b=1, dhi=128(partitions), dho=R, ncn=J and ctx_idx=0 this is
    # exactly "partition p's free row (r j) -> DRAM rows 4p..4p+3" -- the
    # same scatter the plain DMA did.
    out_wb = out[:].rearrange("(b p o) j -> b p o j", b=1, o=R)  # [1,128,4,64]

    with tile.TileContext(nc) as tc:
        with (
            tc.tile_pool(name="sbuf", bufs=1) as sbuf,
            tc.tile_pool(name="psum", bufs=1, space="PSUM") as psum,
        ):
            xt = sbuf.tile([P, FW], FP16)
            ones = sbuf.tile([P, P], FP16)
            ctx0 = sbuf.tile([P, 1], mybir.dt.int32)
            res = sbuf.tile([P, F], FP16)

            # Single input DMA: consecutive transfers complete far apart
            # (HWDGE occupies 625ns per issue), so one transfer wins.
            nc.sync.dma_start(xt[:], xd[:])
            # Stationary all-ones matrix for the cross-partition sum.
            # Pool is idle and this has no input dependency, so it fully
            # hides under the input DMA latency.
            nc.gpsimd.memset(ones[:], 1.0)
            nc.gpsimd.memset(ctx0[:], 0)

            # Pre-generate the OUTPUT DMA descriptors on the SWDGE ring
            # while the input DMA is still in flight: the prep only reads
            # ctx0 (metadata); the res data dep is deferred to trigger_dma
            # below. This moves the ~1.3us HWDGE/DGE descriptor stage off
            # the critical path -- after Ln only the trigger + transfer +
            # completion-sem remain.
            out_dma_sem = nc.alloc_semaphore("out_wb_dma")
            nc.gpsimd.kv_writeback(
                out_wb,
                res[:].rearrange("p (o b j) -> p o b j", o=R, b=1),
                ctx0[:],
                prepare_only=True,
                sem=out_dma_sem,
            )

            w = xt[:, F : F + R]                  # packed exp(diag)-1, [128, 4]
            zeros = xt[:, F + R + 1 : F + R + 2]  # packed 0.0 column

            # E = exp(x), fp16, split 3+1 row blocks: the matmul chain only
            # needs the LAST block late, so a small second exp lets the PE
            # finish the S accumulation sooner after it lands.
            SPL = 3 * J  # 192
            E = sbuf.tile([P, F], FP16)
            nc.scalar.activation(E[:, 0:SPL], xt[:, 0:SPL], Exp, bias=zeros)
            nc.scalar.activation(E[:, SPL:F], xt[:, SPL:F], Exp, bias=zeros)

            # B[m, j] = S[j] for all m: ones.T @ E accumulated over row
            # blocks (fp16 runs the PE at 1 cycle/row).
            B = psum.tile([P, J], FP32)
            for t in range(R):
                nc.tensor.matmul(
                    B[:],
                    ones[:],
                    E[:, ts(t, J)],
                    start=(t == 0),
                    stop=(t == R - 1),
                )

            # E2 = E * w: within row-block r the weight w[4p+r] is a
            # per-partition scalar, so four tensor_scalar ops (64 cols
            # each) fold the multiply without materializing a broadcast
            # weight tile. Blocks 0/1 start right after the first exp.
            # tensor_scalar needs the scalar in f32; Pool widens it while
            # ACT is still on the first exp.
            w32 = sbuf.tile([P, R], FP32)
            nc.gpsimd.tensor_copy(w32[:], w)
            E2 = sbuf.tile([P, F], FP16)
            for t in range(R):
                nc.vector.tensor_scalar(
                    E2[:, ts(t, J)],
                    E[:, ts(t, J)],
                    w32[:, t : t + 1],
                    None,
                    op0=mybir.AluOpType.mult,
                )

            # tmp = E2 + S. Pool (otherwise idle) adds straight from the
            # PSUM accumulator: same engine-busy cost as a DVE PSUM add,
            # but skips the extra copy hop and its semaphore latency.
            tmp = sbuf.tile([P, F], FP16)
            t3 = tmp[:].rearrange("p (r j) -> p r j", r=R)
            e3 = E2[:].rearrange("p (r j) -> p r j", r=R)
            if ADD_VIA == "dve_copy":
                # GPSIMD cannot touch PSUM on real HW, so DVE itself rounds
                # S to fp16 SBUF and then adds in the packed-2-byte fast
                # mode; same-engine in-order, so no extra semaphore hop.
                Bsb = sbuf.tile([P, J], FP16)
                nc.vector.tensor_copy(Bsb[:], B[:])
                nc.vector.tensor_tensor(
                    t3,
                    e3,
                    Bsb[:, None, :].to_broadcast((P, R, J)),
                    op=mybir.AluOpType.add,
                )
            else:
                nc.vector.tensor_tensor(
                    t3,
                    e3,
                    B[:, None, :].to_broadcast((P, R, J)),
                    op=mybir.AluOpType.add,
                )

            # out = log(tmp); single full-width Ln, then fire the
            # pre-generated writeback descriptors. The prep was emitted
            # before res had any producer, so Tile cannot defer the res
            # RAW edge to the trigger on its own; signals_writable=[res]
            # marks res as trigger-accessed, which orders the trigger
            # after the Ln write.
            nc.scalar.activation(res[:], tmp[:], Ln, bias=zeros)
            nc.gpsimd.trigger_dma(count=None, signals_writable=[res[:]])

    _retarget_writeback_sem(nc)
    _strip_spurious_war_guards(nc)
    _strip_post_clear_barrier(nc)
    nc.compile()
    return nc


_NC_CACHE = None


def _pack_inputs(x: np.ndarray, diag: np.ndarray) -> list[dict[str, np.ndarray]]:
    w = (np.exp(diag.astype(np.float64)) - 1.0).astype(np.float16)
    w_blocks = w.reshape(P, R)  # w[4p + r]
    x16 = x.astype(np.float16)
    in_maps = []
    for c in range(N_CORES):
        shard = x16[:, c * J : (c + 1) * J]          # [512, 64]
        xd = np.empty((P, FW), dtype=np.float16)
        xd[:, 0:F] = shard.reshape(P, F)             # rows 4p..4p+3 -> partition p
        xd[:, F : F + R] = w_blocks
        xd[:, F + R] = 1.0
        xd[:, F + R + 1] = 0.0
        in_maps.append({"xd": xd})
    return in_maps


def kernel(x: np.ndarray, diag: np.ndarray, trace: bool = False):
    global _NC_CACHE
    if _NC_CACHE is None:
        _NC_CACHE = build_kernel()
    nc = _NC_CACHE

    x = np.ascontiguousarray(np.asarray(x, dtype=np.float32))
    diag = np.asarray(diag, dtype=np.float32)

    in_maps = _pack_inputs(x, diag)
    res = run_bass_kernel_spmd(nc, in_maps, core_ids=list(range(N_CORES)), trace=trace)
    full = np.concatenate(
        [r["out"].astype(np.float32) for r in res.results], axis=1
    )
    if trace:
        return full, res
    return full


# revision 30
# speedup vs baseline: 1.3151x; 1.0212x over previous
"""Trainium2 Bass kernel for nn_DiagonalMatrixModel.

Reference computes out[i, j] = logsumexp_k(A[i, k] + x[k, j]) with
A = diag(d): a dense log-domain matmul with a diagonal left operand.
Because A[i, k] = d[i] if k == i else 0, the logsumexp collapses exactly:

    out[i, j] = log( sum_{k != i} exp(x[k, j]) + exp(d[i] + x[i, j]) )
              = log( S[j] + exp(x[i, j]) * w[i] ),   w = exp(d) - 1,
    S[j] = sum_k exp(x[k, j])

i.e. O(N^2) work instead of the reference's O(N^3). w is a pure
transform of the learned parameter d, so it is folded on the host
(standard weight preprocessing), keeping the device path x -> out.

Sharding: x and out are split along the column axis j across 8 cores
(64 columns each); w is replicated. Each core computes its S[j]
locally -- no cross-device communication.

Per-core layout: the [512, 64] column shard is viewed as [128, 256]
(partition p holds rows 4p..4p+3); w[4p:4p+4] plus a 0.0 constant are
packed into the same host-side buffer, so each partition's input bytes
are contiguous and ONE DMA fetches everything. The whole on-chip path
runs in fp16 (relative tolerance is 2e-2; fp16 keeps it ~1e-3): this
halves both DMA transfers and lets the DVE run in its packed-2-byte
fast mode and the PE at the 1-cycle/row fp16 rate. The cross-partition
sum S is computed on the tensor engine with an all-ones stationary
matrix, broadcasting S across all 128 partitions of PSUM for free; the
per-row weight multiply is four tensor_scalar ops (w[4p+r] is a
per-partition scalar within each row-block r), and Pool (otherwise
idle) rounds S to an SBUF fp16 copy so the final add runs in the DVE
fast mode too.
"""

import types

import numpy as np

import bass_rust
import concourse.bacc as bacc
import concourse.bass as bass
import concourse.mybir as mybir
from concourse import tile
from concourse.bass import ts
from concourse.bass_utils import run_bass_kernel_spmd
from concourse.hw_specs import get_activation_tables

N_CORES = 8
SIZE = 512          # rows (k / i axis)
N_COLS = 512        # full column count
J = N_COLS // N_CORES  # columns per core
P = 128             # SBUF partitions
R = SIZE // P       # row blocks per partition (4)
F = R * J           # x free-dim elements per partition (256)
FW = F + R + 2      # packed free dim: x (256) + w (4) + consts 1.0, 0.0
HF = F // 2         # half of the x free dim (128)

FP16 = mybir.dt.float16
FP32 = mybir.dt.float32
Exp = mybir.ActivationFunctionType.Exp
Ln = mybir.ActivationFunctionType.Ln
Copy = mybir.ActivationFunctionType.Copy

# The default act-table chooser greedily picks the first set containing
# each needed function (exp_and_others for Exp, then natural_log for Ln)
# => two ~1.3us LoadActFuncSet ops. natural_log_exp_and_others contains
# every function this kernel uses, so blank out all other sets (keeping
# list positions, which define act_func_set_id) to force ONE table load.
_COMBINED_SET = "natural_log_exp_and_others"


def _patched_insert_act_table_loads(self):
    has_activation = any(
        isinstance(i, mybir.InstActivation)
        for b in self.main_func.blocks
        for i in b.instructions
    )
    if not has_activation:
        return
    all_tables = get_activation_tables(self.m.arch)
    if _COMBINED_SET in all_tables:
        tables = [
            (name, funcs if name == _COMBINED_SET else set())
            for name, funcs in all_tables.items()
        ]
    else:  # safety: unknown act_info layout -> default behavior
        tables = list(all_tables.items())
    bass_rust.insert_act_table_loads(self, tables)


def _strip_const_preamble(nc) -> None:
    """Drop the const-AP preamble: the 4 memsets and the all-engine
    barrier that publishes them. This kernel passes its own zeros tile as
    the activation bias, so no const AP is ever read. Saves ~600ns before
    the input DMA can issue."""
    bb = nc.main_func.blocks[0]
    dead = [
        ins
        for ins in bb.instructions
        if type(ins).__name__ in ("InstMemset", "InstDrain", "InstEventSemaphore")
    ]
    for ins in dead:
        bb.instructions.remove(ins)


def _diet_tail(nc) -> None:
    """Slim the kernel-exit path.

    (1) The SP kernel-tail drain waits, one sequencer step at a time, on
    every engine/queue sem -- all of which are long satisfied when the
    output-DMA completion (DMASW*) finally lands. Keep only the DMASW
    waits; the gather barrier already proves the engines drained.

    (2) Each non-Pool engine ends with a release-barrier wait whose only
    effect is to delay stream-end until after Pool's sem-clear STARTS.
    NEFF completion requires every stream to end, and Pool ends after the
    clear either way, so dropping the release waiters changes nothing for
    either a single run or re-execution."""
    keep_prefixes = ("DMASW",)
    blocks = list(nc.main_func.blocks)
    trig_block = max(
        (
            bi
            for bi, bb in enumerate(blocks)
            for ins in bb.instructions
            if type(ins).__name__ == "InstTriggerDma"
        ),
        default=None,
    )
    if trig_block is None:
        return
    for bi, bb in enumerate(blocks):
        if bi <= trig_block:
            trig = [
                i
                for i, ins in enumerate(bb.instructions)
                if type(ins).__name__ == "InstTriggerDma"
            ]
            if not trig:
                continue
            region = bb.instructions[trig[-1] + 1 :]
        else:
            region = list(bb.instructions)
        dead = []
        for ins in region:
            si = getattr(ins, "sync_info", None)
            if not si:
                continue
            tn = type(ins).__name__
            if tn in ("InstDrain", "InstEventSemaphore") and not si.on_update:
                ws = si.on_wait
                if ws and all(
                    w.ant_name
                    and (
                        w.ant_name.endswith("_49")
                        or w.ant_name.startswith("DMA")
                        or "sequencer" in w.ant_name
                    )
                    for w in ws
                ):
                    kept = [
                        w
                        for w in ws
                        if w.ant_name and w.ant_name.startswith(keep_prefixes)
                    ]
                    if len(kept) != len(ws):
                        if kept or tn == "InstDrain":
                            si.on_wait = kept
                        else:
                            dead.append(ins)
            # release-barrier waiters on non-Pool engines
            name = getattr(ins, "name", "")
            if (
                tn == "InstEventSemaphore"
                and isinstance(name, str)
                and name.startswith("barrier_")
                and not name.startswith("barrier_Pool")
                and any(
                    w.ant_name and w.ant_name.endswith("_release") for w in si.on_wait
                )
            ):
                dead.append(ins)
        for ins in dead:
            bb.instructions.remove(ins)


def _strip_post_clear_barrier(nc) -> None:
    """Drop the all-engine barrier emitted AFTER the kernel-tail semaphore
    clear. NEFF completion requires every engine stream to end, and the
    Pool sem-clear is Pool's last instruction either way, so the barrier
    only delays stream-end by ~300ns. Sem state for re-execution is
    unchanged (the clear itself is kept, ordered after the pre-clear
    barrier)."""
    bb = nc.main_func.blocks[-1]
    isa_idx = max(
        (i for i, ins in enumerate(bb.instructions)
         if type(ins).__name__ == "InstISA"),
        default=None,
    )
    if isa_idx is None:
        return
    tail = bb.instructions[isa_idx + 1 :]
    if not all(
        type(ins).__name__ in ("InstDrain", "InstEventSemaphore") for ins in tail
    ):
        return  # unexpected tail layout -> leave it intact
    for ins in tail:
        bb.instructions.remove(ins)


# Add variant: "dve_copy" = DVE copies S to SBUF fp16 then adds in fast
# mode (in-order, no extra sem hop); "psum" = DVE adds the PSUM f32
# accumulator directly in one slower op.
ADD_VIA = "psum"
# Number of exp chunks: 1 = single ACT op (latest first-sem but least ACT
# busy), 2 = 3+1 row-block split.
EXP_SPLIT = 2
# Dummy warm-up matmuls to hold the PE at a ramped p-state before the
# real accumulation (0 = off).
PE_WARMERS = 0
# Split the final Ln (and the writeback) into halves with separate
# triggers so the two 900ns completion props overlap.
LN_SPLIT = False


def _retarget_writeback_sem(nc) -> None:
    """Point the kv_writeback prep's DMA-completion update at the builtin
    DMASW0 queue semaphore. Tile schedules the prep on the DMASW0 proc lane
    and makes downstream waiters (the kernel-tail barriers) wait
    DMASW0 >= 16, but the descriptor-baked sem comes from the user `sem=`
    kwarg -- without this rewrite the completion bumps the wrong sem and
    the tail deadlocks."""
    lanes = {}
    for bb in nc.main_func.blocks:
        for ins in bb.instructions:
            si = getattr(ins, "sync_info", None)
            if not si:
                continue
            for w in si.on_wait:
                if w.ant_name and w.ant_name.startswith("DMASW"):
                    lane = int(w.ant_name[len("DMASW") :].split("_")[0])
                    lanes[lane] = (w.id, w.ant_name)
    assert lanes, "no DMASW waiter found"
    preps = [
        ins
        for bb in nc.main_func.blocks
        for ins in bb.instructions
        if type(ins).__name__ == "InstKVWritebackAnt"
    ]
    assert len(preps) == len(lanes), (len(preps), lanes)
    for i, prep in enumerate(preps):
        upd = prep.sync_info.on_update[0]
        assert upd.ant_name == "out_wb_dma", upd.ant_name
        upd.id, upd.ant_name = lanes[i]


def _strip_spurious_war_guards(nc) -> None:
    """Remove the write-after-read guards Tile places before the Ln and the
    trigger. The kv_writeback prep is emitted before res has a producer, so
    Tile models the prep's deferred res-read as completing at DMASW0>=16 and
    makes the later res writer (Ln) -- and even the trigger itself -- wait
    for it. The DMA only fires at the trigger, which already waits on the
    Ln via signals_writable, so these guards are a false cycle: the real
    ordering Ln -> trigger -> DMA is intact without them. The SP kernel-tail
    gate (which also waits DMASW0>=16, together with other sems) is kept --
    it is what holds the NEFF open until the output lands in DRAM."""
    for bb in nc.main_func.blocks:
        for ins in bb.instructions:
            if type(ins).__name__ not in (
                "InstActivation",
                "InstTriggerDma",
                "InstKVWritebackAnt",
            ):
                continue
            si = getattr(ins, "sync_info", None)
            if not si:
                continue
            kept = [
                w
                for w in si.on_wait
                if not (w.ant_name and w.ant_name.startswith("DMASW"))
            ]
            if len(kept) != len(si.on_wait):
                si.on_wait = kept


def build_kernel() -> bass.Bass:
    nc = bacc.Bacc("TRN2")
    nc.insert_act_table_loads = types.MethodType(_patched_insert_act_table_loads, nc)
    _strip_const_preamble(nc)

    xd = nc.dram_tensor("xd", [P, FW], FP16, kind="ExternalInput")
    out = nc.dram_tensor("out", [SIZE, J], FP16, kind="ExternalOutput")
    # kv_writeback layout: dst[b, dhi, dho, ctx:ctx+ncn] = src[dhi, dho, b, :].
    # With b=1, dhi=128(partitions), dho=R, ncn=J and ctx_idx=0 this is
    # exactly "partition p's free row (r j) -> DRAM rows 4p..4p+3" -- the
    # same scatter the plain DMA did.
    out_wb = out[:].rearrange("(b p o) j -> b p o j", b=1, o=R)  # [1,128,4,64]

    with tile.TileContext(nc) as tc:
        with (
            tc.tile_pool(name="sbuf", bufs=1) as sbuf,
            tc.tile_pool(name="psum", bufs=1, space="PSUM") as psum,
        ):
            xt = sbuf.tile([P, FW], FP16)
            ones = sbuf.tile([P, P], FP16)
            ctx0 = sbuf.tile([P, 1], mybir.dt.int32)
            res = sbuf.tile([P, F], FP16)

            # Single input DMA: consecutive transfers complete far apart
            # (HWDGE occupies 625ns per issue), so one transfer wins.
            nc.sync.dma_start(xt[:], xd[:])
            # Stationary all-ones matrix for the cross-partition sum.
            # Pool is idle and this has no input dependency, so it fully
            # hides under the input DMA latency.
            nc.gpsimd.memset(ones[:], 1.0)
            nc.gpsimd.memset(ctx0[:], 0)

            # Pre-generate the OUTPUT DMA descriptors on the SWDGE ring
            # while the input DMA is still in flight: the prep only reads
            # ctx0 (metadata); the res data dep is deferred to trigger_dma
            # below. This moves the ~1.3us HWDGE/DGE descriptor stage off
            # the critical path -- after Ln only the trigger + transfer +
            # completion-sem remain.
            out_dma_sem = nc.alloc_semaphore("out_wb_dma")
            if LN_SPLIT:
                # Two half-writebacks placed via ctx_idx (0 and HF along a
                # 256-wide n_ctx) so each can fire right after its Ln half
                # and the two 900ns completion props overlap.
                ctxh = sbuf.tile([P, 1], mybir.dt.int32)
                nc.gpsimd.memset(ctxh[:], HF)
                out_flat = out[:].rearrange("(b p o) j -> b p o (j)", b=1, o=R)
                out_full = out[:].rearrange("(b p) (o j) -> b p o j", b=1, o=1)
                prep_sem = nc.alloc_semaphore("out_wb_prep")
                for h, ctx_t in ((0, ctx0), (1, ctxh)):
                    nc.gpsimd.kv_writeback(
                        out_full,
                        res[:, h * HF : (h + 1) * HF].rearrange(
                            "p (o b j) -> p o b j", o=1, b=1
                        ),
                        ctx_t[:],
                        prepare_only=True,
                        sem=out_dma_sem,
                    ).then_inc(prep_sem, 1)
            else:
                nc.gpsimd.kv_writeback(
                    out_wb,
                    res[:].rearrange("p (o b j) -> p o b j", o=R, b=1),
                    ctx0[:],
                    prepare_only=True,
                    sem=out_dma_sem,
                )

            if PE_WARMERS:
                # Keep the PE p-state ramped so the real accumulation runs
                # at the warm rate instead of the cold 1.54 cycles/row.
                scratch = psum.tile([P, J], FP32)
                for _ in range(PE_WARMERS):
                    nc.tensor.matmul(
                        scratch[:], ones[:], ones[:, 0:J], start=True, stop=True
                    )

            w = xt[:, F : F + R]                  # packed exp(diag)-1, [128, 4]
            zeros = xt[:, F + R + 1 : F + R + 2]  # packed 0.0 column

            # E = exp(x), fp16. EXP_SPLIT=2 splits 3+1 row blocks (the
            # matmul chain only needs the last block late); 1 runs one op
            # (~190ns less ACT busy, but everything waits the single sem).
            E = sbuf.tile([P, F], FP16)
            if EXP_SPLIT == 1:
                nc.scalar.activation(E[:], xt[:, 0:F], Exp, bias=zeros)
            else:
                SPL = 3 * J  # 192
                nc.scalar.activation(E[:, 0:SPL], xt[:, 0:SPL], Exp, bias=zeros)
                nc.scalar.activation(E[:, SPL:F], xt[:, SPL:F], Exp, bias=zeros)

            # B[m, j] = S[j] for all m: ones.T @ E accumulated over row
            # blocks (fp16 runs the PE at 1 cycle/row).
            B = psum.tile([P, J], FP32)
            for t in range(R):
                nc.tensor.matmul(
                    B[:],
                    ones[:],
                    E[:, ts(t, J)],
                    start=(t == 0),
                    stop=(t == R - 1),
                )

            # E2 = E * w: within row-block r the weight w[4p+r] is a
            # per-partition scalar, so four tensor_scalar ops (64 cols
            # each) fold the multiply without materializing a broadcast
            # weight tile. Blocks 0/1 start right after the first exp.
            # tensor_scalar needs the scalar in f32; Pool widens it while
            # ACT is still on the first exp.
            w32 = sbuf.tile([P, R], FP32)
            nc.gpsimd.tensor_copy(w32[:], w)
            E2 = sbuf.tile([P, F], FP16)
            for t in range(R):
                nc.vector.tensor_scalar(
                    E2[:, ts(t, J)],
                    E[:, ts(t, J)],
                    w32[:, t : t + 1],
                    None,
                    op0=mybir.AluOpType.mult,
                )

            # tmp = E2 + S. Pool (otherwise idle) adds straight from the
            # PSUM accumulator: same engine-busy cost as a DVE PSUM add,
            # but skips the extra copy hop and its semaphore latency.
            tmp = sbuf.tile([P, F], FP16)
            t3 = tmp[:].rearrange("p (r j) -> p r j", r=R)
            e3 = E2[:].rearrange("p (r j) -> p r j", r=R)
            if ADD_VIA == "stt_copy":
                # DVE rounds S into fp16 SBUF, then adds in the packed
                # fast mode. The copy is phrased as scalar_tensor_tensor
                # with a dummy bypass read of E2's last block: that real
                # RAW edge pins it AFTER the tensor_scalar ops in the
                # DVE queue (the Tile scheduler otherwise hoists the copy
                # to the front, where its PE wait stalls the whole FIFO).
                Bsb = sbuf.tile([P, J], FP16)
                nc.vector.scalar_tensor_tensor(
                    Bsb[:],
                    B[:],
                    1.0,
                    E2[:, ts(R - 1, J)],
                    op0=mybir.AluOpType.bypass,
                    op1=mybir.AluOpType.bypass,
                )
                nc.vector.tensor_tensor(
                    t3,
                    e3,
                    Bsb[:, None, :].to_broadcast((P, R, J)),
                    op=mybir.AluOpType.add,
                )
            elif ADD_VIA == "dve_copy":
                # GPSIMD cannot touch PSUM on real HW, so DVE itself rounds
                # S to fp16 SBUF and then adds in the packed-2-byte fast
                # mode; same-engine in-order, so no extra semaphore hop.
                Bsb = sbuf.tile([P, J], FP16)
                nc.vector.tensor_copy(Bsb[:], B[:])
                nc.vector.tensor_tensor(
                    t3,
                    e3,
                    Bsb[:, None, :].to_broadcast((P, R, J)),
                    op=mybir.AluOpType.add,
                )
            else:
                nc.vector.tensor_tensor(
                    t3,
                    e3,
                    B[:, None, :].to_broadcast((P, R, J)),
                    op=mybir.AluOpType.add,
                )

            # out = log(tmp), then fire the pre-generated writeback
            # descriptors. The prep was emitted before res had any
            # producer, so Tile cannot defer the res RAW edge to the
            # trigger on its own; signals_writable=[res] marks res as
            # trigger-accessed, which orders the trigger after the Ln
            # write.
            if LN_SPLIT:
                nc.gpsimd.wait_ge(prep_sem, 2)
                for h in range(2):
                    sl = slice(h * HF, (h + 1) * HF)
                    nc.scalar.activation(res[:, sl], tmp[:, sl], Ln, bias=zeros)
                    nc.gpsimd.trigger_dma(
                        count=1, signals_writable=[res[:, sl]]
                    )
            else:
                nc.scalar.activation(res[:], tmp[:], Ln, bias=zeros)
                nc.gpsimd.trigger_dma(count=None, signals_writable=[res[:]])

    _retarget_writeback_sem(nc)
    _strip_spurious_war_guards(nc)
    _diet_tail(nc)
    _strip_post_clear_barrier(nc)
    nc.compile()
    return nc


_NC_CACHE = None


def _pack_inputs(x: np.ndarray, diag: np.ndarray) -> list[dict[str, np.ndarray]]:
    w = (np.exp(diag.astype(np.float64)) - 1.0).astype(np.float16)
    w_blocks = w.reshape(P, R)  # w[4p + r]
    x16 = x.astype(np.float16)
    in_maps = []
    for c in range(N_CORES):
        shard = x16[:, c * J : (c + 1) * J]          # [512, 64]
        xd = np.empty((P, FW), dtype=np.float16)
        xd[:, 0:F] = shard.reshape(P, F)             # rows 4p..4p+3 -> partition p
        xd[:, F : F + R] = w_blocks
        xd[:, F + R] = 1.0
        xd[:, F + R + 1] = 0.0
        in_maps.append({"xd": xd})
    return in_maps


def kernel(x: np.ndarray, diag: np.ndarray, trace: bool = False):
    global _NC_CACHE
    if _NC_CACHE is None:
        _NC_CACHE = build_kernel()
    nc = _NC_CACHE

    x = np.ascontiguousarray(np.asarray(x, dtype=np.float32))
    diag = np.asarray(diag, dtype=np.float32)

    in_maps = _pack_inputs(x, diag)
    res = run_bass_kernel_spmd(nc, in_maps, core_ids=list(range(N_CORES)), trace=trace)
    full = np.concatenate(
        [r["out"].astype(np.float32) for r in res.results], axis=1
    )
    if trace:
        return full, res
    return full
